# revision 20
# baseline (speedup 1.0000x reference)
"""Trainium2 Bass kernel for BaseTensorMemory (delta-rule tensor memory).

Computes, for full inputs queries/keys/values [B,S,D], M [D,D], z [D]:
  sigma_k = elu(keys)+1 ; existing = (sigma_k@M)/(sigma_k@z+eps)
  delta_m = clip(einsum('bsd,bse->de', sigma_k, values-existing)/(B*S), +-1)
  delta_z = sigma_k.sum((0,1))/B
  M' = clip(M+delta_m, +-100); z' = clip(z+delta_z, eps, 1e6)
  out = (sigma_q@M')/(sigma_q@z'+eps)

Strategy: data-parallel over 8 NeuronCores. Flatten B*S tokens, shard
contiguously. Per core: keys pass accumulates [sigma^T@v | sigma^T@1 |
sigma^T@ex] into PSUM, AllReduce the tiny [64,129] partial, build M'/z'
on-chip, then the queries retrieve pass streams the output (bf16).

Engine balance (the point of this version): every [128,1024] bulk op is
placed so ACT / DVE / PE each stay near the ~2.9us/macro-tile DMA floor.
GPSIMD bulk elementwise is ~12x slower than DVE and poisons concurrent
DVE throughput (measured) — it only does tiny memsets, small DMAs, the
casting values->vox DMA (SWDGE descriptor gen) and the collective.
  keys:    ACT exp + 2 sigT copies | DVE min(e,1) (4x) + relu-fold stt +
           ex-mul + recip | PE trans/retr/delta | vox via casting DMA
  queries: ACT exp + relu + 1 sigT copy | DVE min(e,1) + out-mul + recip
           + 1 sigT copy | PE sums min(e,1).T + relu.T via accumulating
           regular matmuls (queries never need token-major sigma)
Other cuts vs the previous version: +eps dropped from the reciprocal
(norm >= ~5, error <= 2e-7), negation folded into the post-allreduce
subtract, output written bf16 (f16 would denorm: out absmax ~3.7e-5).

elu(x)+1 == min(exp(x), relu(x)+1) exactly (e^x >= x+1 everywhere).

Device quirk found empirically: matmuls whose operands alternate base
partition (0 vs 64) inside one PSUM accumulation group hard-crash the
device (NRT_EXEC_UNIT_UNRECOVERABLE). Retrieval is therefore parity-banked:
even token-groups accumulate in bank 0 (operands at base partition 0), odd
groups in bank 1 (base partition 64).
"""

import numpy as np

B, S, D = 16, 16384, 64
N_CORES = 8
EPS = 1e-6
MAX_DELTA = 1.0
MAX_MEMORY = 100.0
MAX_NORM = 1e6

TILE_TOKENS = 2048  # macro-tile: [128, 1024] f32, two 1024-token halves
QPM = TILE_TOKENS // 128  # 16 token-groups per macro-tile
VW = 2 * D + 1  # 129: [v | ones | ex] block per group in VOX


def _build(n_cores, tokens_per_core):
    import concourse.bacc as bacc
    import concourse.mybir as mybir
    import concourse.tile as tile
    from concourse import masks

    dt = mybir.dt
    f32, f16, bf16 = dt.float32, dt.float16, dt.bfloat16
    A = mybir.AluOpType
    F = mybir.ActivationFunctionType

    T = tokens_per_core
    NT = T // TILE_TOKENS
    assert NT * TILE_TOKENS == T
    MFD = QPM * D  # 1024: macro-tile free dim

    nc = bacc.Bacc(
        "TRN2", target_bir_lowering=False, debug=False, num_devices=n_cores
    )
    k_d = nc.dram_tensor("keys", [T, D], f32, kind="ExternalInput").ap()
    v_d = nc.dram_tensor("values", [T, D], f32, kind="ExternalInput").ap()
    q_d = nc.dram_tensor("queries", [T, D], f32, kind="ExternalInput").ap()
    m_d = nc.dram_tensor("m", [D, D], f32, kind="ExternalInput").ap()
    z_d = nc.dram_tensor("z", [D, 1], f32, kind="ExternalInput").ap()
    o_d = nc.dram_tensor("out", [T, D], bf16, kind="ExternalOutput").ap()

    kr = k_d.rearrange("(n p q) d -> n p (q d)", p=128, q=QPM)
    vr = v_d.rearrange("(n p q) d -> n p (q d)", p=128, q=QPM)
    qr = q_d.rearrange("(n p q) d -> n p (q d)", p=128, q=QPM)
    orr = o_d.rearrange("(n p q) d -> n p (q d)", p=128, q=QPM)

    with tile.TileContext(nc) as tc:
        with (
            tc.tile_pool(name="const", bufs=1) as cpool,
            tc.tile_pool(name="io", bufs=4) as io,
            tc.tile_pool(name="qio", bufs=6) as qio,
            tc.tile_pool(name="work", bufs=3) as work,
            tc.tile_pool(name="small", bufs=6) as small,
            tc.tile_pool(name="sigq", bufs=20) as sigq,
            tc.tile_pool(name="psR", bufs=2, space="PSUM") as psRp,
            tc.tile_pool(name="psA", bufs=1, space="PSUM") as psAp,
            tc.tile_pool(name="dram", bufs=1, space="DRAM") as dram,
        ):
            # issue the first macro-tile's loads before anything else so the
            # ACT exp can start as early as possible
            kt0 = io.tile([128, MFD], f32, tag="kt")
            nc.sync.dma_start(kt0[:], kr[0])

            ident = cpool.tile([128, 128], f16)
            masks.make_identity(nc, ident[:])

            # [M|z] in fp16, replicated on both partition halves (parity-
            # banked retrieve uses operands at base partition 0 and 64).
            mz = cpool.tile([128, 65], f32)
            nc.sync.dma_start(mz[0:64, 0:64], m_d[:])
            nc.sync.dma_start(mz[0:64, 64:65], z_d[:])
            nc.sync.dma_start(mz[64:128, 0:64], m_d[:])
            nc.sync.dma_start(mz[64:128, 64:65], z_d[:])
            mz16 = cpool.tile([128, 65], f16)
            nc.scalar.copy(mz16[:], mz[:])

            psA = psAp.tile([64, VW], f32)
            psTh = [None]  # current phase's psT pool

            def elu_k_tile(kt):
                """keys sigma = min(exp(x),1) + relu(x) == elu(x)+1 exactly
                (e^x >= x+1 everywhere). Two DVE ops, no relu pass:
                ep1 = min(e,1) at 4x mode, then one 1x stt folds the relu:
                sig = (x max 0) + ep1. GPSIMD bulk ops are ~12x slower than
                DVE and poison concurrent DVE throughput — measured, avoid."""
                e16 = work.tile([128, MFD], f16, tag="e")
                nc.scalar.activation(e16[:], kt[:], F.Exp)
                ep1 = work.tile([128, MFD], f16, tag="ep1")
                nc.vector.tensor_scalar_min(ep1[:], e16[:], 1.0)
                sig = work.tile([128, MFD], f16, tag="sig")
                nc.vector.scalar_tensor_tensor(
                    sig[:], kt[:], 0.0, ep1[:], op0=A.max, op1=A.add
                )
                return sig

            def elu_q_tile(qt):
                """queries sigma pieces: ep1 = min(exp(x),1) (DVE 4x),
                r16 = relu(x) (ACT). Queries never need token-major sigma,
                only sigma^T — the two pieces are summed for free on the PE
                by accumulating two REGULAR matmuls against identity (the
                is_transpose path can NOT accumulate — it overwrites PSUM
                regardless of start/stop, found empirically — but regular
                matmuls accumulate fine and lhsT.T @ I is a transpose)."""
                e16 = work.tile([128, MFD], f16, tag="e")
                nc.scalar.activation(e16[:], qt[:], F.Exp)
                r16 = work.tile([128, MFD], f16, tag="r")
                nc.scalar.activation(r16[:], qt[:], F.Relu)
                ep1 = work.tile([128, MFD], f16, tag="ep1")
                nc.vector.tensor_scalar_min(ep1[:], e16[:], 1.0)
                return ep1, r16

            def transpose_half(sig, a, copy_eng):
                """4x [128,128] PE transposes of half a -> sigT [128, 512]
                f16 (token-groups parity-stacked on partitions). start=True
                lazily zeroes the whole PSUM bank: first matmul only.
                The psT pool is phase-scoped (keys f16 / queries f32
                pools reuse the same PSUM banks: 3 + 2x2 + 1 = 8)."""
                psT = psTh[0].tile([128, 512], f16, tag="psT")
                for c in range(4):
                    nc.tensor.matmul(
                        psT[:, c * 128 : (c + 1) * 128],
                        sig[:, a * 512 + c * 128 : a * 512 + (c + 1) * 128],
                        ident[:],
                        is_transpose=True,
                        start=(c == 0),
                        stop=(c == 3),
                    )
                sigT = sigq.tile([128, 512], f16, tag="sigT" + copy_eng)
                if copy_eng == "act":
                    nc.scalar.copy(sigT[:], psT[:])
                else:
                    nc.vector.tensor_copy(sigT[:], psT[:])
                return sigT

            def transpose_sum_half(ep1, r16, a, copy_eng):
                """sigT = (ep1 + r16).T via accumulating REGULAR matmuls:
                chunk.T @ I accumulated in PSUM over the two source tiles.
                start=True lazily zeroes the whole PSUM bank: first mm
                only. Regular matmuls must write f32 PSUM."""
                psT = psTh[0].tile([128, 512], f32, tag="psTs")
                for c in range(4):
                    for t_idx, src_t in enumerate((r16, ep1)):
                        nc.tensor.matmul(
                            psT[:, c * 128 : (c + 1) * 128],
                            src_t[:, a * 512 + c * 128 : a * 512 + (c + 1) * 128],
                            ident[:],
                            start=(c == 0 and t_idx == 0),
                            stop=(c == 3 and t_idx == 1),
                        )
                sigT = sigq.tile([128, 512], f16, tag="sigT" + copy_eng)
                if copy_eng == "act":
                    nc.scalar.copy(sigT[:], psT[:])
                else:
                    nc.vector.tensor_copy(sigT[:], psT[:])
                return sigT

            def retrieve_half(sigT, mztile):
                """Parity-banked: even groups -> bank0 (base 0), odd ->
                bank1 (base 64). In-half group (h par, j) = 2j+h at bank h
                col 65j. Returns psum + data/norm views [128, h, j, *]."""
                psR = psRp.tile([128, 1024], f32, tag="psR")
                for g in range(8):
                    par, j = g % 2, g // 2
                    off = par * 512 + 65 * j
                    base = par * 64
                    lhsT = sigT[base : base + 64, j * 128 : (j + 1) * 128]
                    nc.tensor.matmul(
                        psR[:, off : off + 65],
                        lhsT,
                        mztile[base : base + 64, :],
                        start=(j == 0),
                        stop=(j == 3),
                    )
                ret = psR[:].rearrange("p (h x) -> p h x", h=2)[:, :, 0:260]
                ret = ret.rearrange("p h (j c) -> p h j c", j=4)
                return psR, ret[:, :, :, 0:64], ret[:, :, :, 64:65]

            def recip_norm(normv):
                """1/norm broadcast view [128, h, j, 64] f32 (eps dropped:
                norm >= ~5 in practice, error <= 2e-7)."""
                rn = small.tile([128, 8], f32, tag="rn")
                rnv = rn[:].rearrange("p (h j) -> p h j", h=2).unsqueeze(3)
                nc.vector.reciprocal(rnv, normv)
                return rnv.broadcast_to((128, 2, 4, 64))

            # ---------------- keys phase ----------------
            # Stage pipeline over 1024-token halves:
            #   front(h): elu + transposes + sigT copy
            #   mid(h):   retrieve + recip + ex-mul   (1 half behind front)
            #   back(h):  8 delta matmuls             (2 halves behind)
            NH = 2 * NT
            fronts = {}
            first_mm = [True]

            def front_k(h):
                i, a = h // 2, h % 2
                if a == 0:
                    if i == 0:
                        kt = kt0
                    else:
                        # k/v/q arrive f16 via contiguous casting DMAs —
                        # halves SBUF-side DMA traffic; exp(x*(1+2^-11))
                        # keeps sigma within ~0.3% (gate is 2e-2). A strided
                        # casting DMA straight into vox writes 128B chunks —
                        # below the 512B SDMA line-rate threshold — and
                        # saturates the DMA engines (measured); the f16
                        # interleave into vox runs on ACT instead.
                        kt = io.tile([128, MFD], f16, tag="kt")
                        nc.gpsimd.dma_start(kt[:], kr[i])
                    sig = elu_k_tile(kt)
                    vt16 = io.tile([128, MFD], f16, tag="vt16")
                    nc.gpsimd.dma_start(vt16[:], vr[i])
                    vox = work.tile([128, QPM * VW], f16, tag="vox")
                    voxg = vox[:].rearrange("p (g c) -> p g c", g=QPM)
                    nc.scalar.copy(voxg[:, :, 0:64], vt16[:])
                    nc.gpsimd.memset(voxg[:, :, 64:65], 1.0)
                    fronts[i] = (sig, vox)
                sig, vox = fronts[i]
                sigT = transpose_half(sig, a, copy_eng="act")
                return sigT

            def mid_k(h, sigT):
                i, a = h // 2, h % 2
                sig, vox = fronts[i]
                psR, data, normv = retrieve_half(sigT, mz16)
                rn = recip_norm(normv)
                exv = vox[:, a * 8 * VW : (a + 1) * 8 * VW].rearrange(
                    "p (j h c) -> p h j c", j=4, h=2
                )[:, :, :, 65:129]
                nc.vector.tensor_mul(exv, data, rn)

            def back_k(h, last):
                i, a = h // 2, h % 2
                sig, vox = fronts[i]
                for g in range(8):
                    q = a * 8 + g
                    nc.tensor.matmul(
                        psA[:],
                        sig[:, q * 64 : (q + 1) * 64],
                        vox[:, q * VW : (q + 1) * VW],
                        start=first_mm[0],
                        stop=(last and g == 7),
                    )
                    first_mm[0] = False

            stages = []
            for h in range(NH):
                stages.append(("f", h))
                if h >= 1:
                    stages.append(("m", h - 1))
                if h >= 2:
                    stages.append(("b", h - 2))
            stages += [("m", NH - 1), ("b", NH - 2), ("b", NH - 1)]
            sigTs = {}
            with tc.tile_pool(name="psTk", bufs=3, space="PSUM") as psTkp:
                psTh[0] = psTkp
                for kind, h in stages:
                    if kind == "f":
                        sigTs[h] = front_k(h)
                    elif kind == "m":
                        mid_k(h, sigTs[h])
                    else:
                        back_k(h, last=(h == NH - 1))

            # ---------------- allreduce + update ----------------
            accsb = cpool.tile([64, VW], f32)
            nc.vector.tensor_copy(accsb[:], psA[:])
            if n_cores > 1:
                arin = dram.tile([64, VW], f32)
                arout = dram.tile([64, VW], f32)
                nc.gpsimd.dma_start(arin[:], accsb[:])
                nc.gpsimd.collective_compute(
                    "AllReduce",
                    mybir.AluOpType.add,
                    replica_groups=[list(range(n_cores))],
                    ins=[arin.opt()],
                    outs=[arout.opt()],
                )
                arsb = cpool.tile([64, VW], f32)
                nc.gpsimd.dma_start(arsb[:], arout[:])
            else:
                arsb = accsb

            def update_math():
                # 6 fused DVE ops; emitted after SKEW queries fronts so the
                # DVE stream has front work while the AllReduce runs.
                # delta_m = clip((sv-sex)/(B*S), +-1); M' = clip(M+dm, +-100)
                nc.vector.tensor_sub(
                    mzn[:, 0:64], arsb[:, 0:64], arsb[:, 65:129]
                )
                nc.vector.tensor_scalar(
                    mzn[:, 0:64], mzn[:, 0:64], 1.0 / (B * S), MAX_DELTA,
                    op0=A.mult, op1=A.min,
                )
                nc.vector.scalar_tensor_tensor(
                    mzn[:, 0:64], mzn[:, 0:64], -MAX_DELTA, mz[0:64, 0:64],
                    op0=A.max, op1=A.add,
                )
                nc.vector.tensor_scalar(
                    mzn[:, 0:64], mzn[:, 0:64], MAX_MEMORY, -MAX_MEMORY,
                    op0=A.min, op1=A.max,
                )
                # delta_z = acc_z/B; z' = clip(z+dz, eps, 1e6)
                nc.vector.scalar_tensor_tensor(
                    mzn[:, 64:65], arsb[:, 64:65], 1.0 / B, mz[0:64, 64:65],
                    op0=A.mult, op1=A.add,
                )
                nc.vector.tensor_scalar(
                    mzn[:, 64:65], mzn[:, 64:65], EPS, MAX_NORM,
                    op0=A.max, op1=A.min,
                )

            mzn = cpool.tile([64, 65], f32)
            mzn128 = cpool.tile([128, 65], f32)
            mzn16 = cpool.tile([128, 65], f16)

            def update_cast():
                nc.gpsimd.dma_start(mzn128[0:64, :], mzn[:])
                nc.gpsimd.dma_start(mzn128[64:128, :], mzn[:])
                nc.gpsimd.tensor_copy(mzn16[:], mzn128[:])

            # ---------------- queries phase ----------------
            outs = {}

            def front_q(h):
                i, a = h // 2, h % 2
                if a == 0:
                    # f16 casting DMA on the GPSIMD queue: q prefetches are
                    # naturally paced behind the keys-phase loads instead of
                    # stealing keys DMA bandwidth from the sync queue.
                    qt = qio.tile([128, MFD], f16, tag="qt")
                    nc.gpsimd.dma_start(qt[:], qr[i])
                    fronts[i + NT] = elu_q_tile(qt)
                ep1, r16 = fronts[i + NT]
                eng = "act" if a == 0 else "dve"
                return transpose_sum_half(ep1, r16, a, copy_eng=eng)

            def mid_q(h):
                i, a = h // 2, h % 2
                psR, data, normv = retrieve_half(sigTs[h], mzn16)
                rn = recip_norm(normv)
                if a == 0:
                    ot = io.tile([128, MFD], bf16, tag="ot")
                    outs[i] = ot
                ot = outs[i]
                otv = ot[:, a * 512 : (a + 1) * 512].rearrange(
                    "p (j h c) -> p h j c", j=4, h=2
                )
                nc.vector.tensor_mul(otv, data, rn)
                if a == 1:
                    nc.sync.dma_start(orr[i], ot[:])

            SKEW = min(14, NH)
            sigTs = {}
            with tc.tile_pool(name="psTq", bufs=3, space="PSUM") as psTqp:
                psTh[0] = psTqp
                for h in range(min(SKEW, NH)):
                    sigTs[h] = front_q(h)
                update_math()
                update_cast()
                for h in range(SKEW, NH):
                    sigTs[h] = front_q(h)
                    mid_q(h - SKEW)
                for h in range(max(0, NH - SKEW), NH):
                    mid_q(h)

    nc.compile()
    return nc


_CACHE = {}


def _get_kernel(n_cores, tokens_per_core):
    key = (n_cores, tokens_per_core)
    if key not in _CACHE:
        _CACHE[key] = _build(n_cores, tokens_per_core)
    return _CACHE[key]


def _np_reference(queries, keys, values, M, z):
    """Fallback (is_empty edge case) — straight numpy port of the reference."""

    def elu1(x):
        return np.where(x > 0, x + 1.0, np.exp(np.minimum(x, 0.0)))

    def retrieve(sig, M, z):
        return (sig @ M) / ((sig @ z)[..., None] + EPS)

    sk = elu1(keys)
    existing = retrieve(sk, M, z)
    uv = values if z.sum() == 0 else values - existing
    dm = np.clip(
        np.einsum("bsd,bse->de", sk, uv) / (B * S), -MAX_DELTA, MAX_DELTA
    )
    dz = sk.sum(axis=(0, 1)) / B
    Mn = np.clip(M + dm, -MAX_MEMORY, MAX_MEMORY)
    zn = np.clip(z + dz, EPS, MAX_NORM)
    return retrieve(elu1(queries), Mn, zn).astype(np.float32)


def kernel(queries, keys, values, M, z, _want_results_obj=False, **_ignored):
    from concourse import bass_utils

    queries = np.ascontiguousarray(queries, dtype=np.float32)
    keys = np.ascontiguousarray(keys, dtype=np.float32)
    values = np.ascontiguousarray(values, dtype=np.float32)
    M = np.ascontiguousarray(M, dtype=np.float32)
    z = np.ascontiguousarray(z, dtype=np.float32)

    if float(z.sum()) == 0.0:
        # is_empty branch of the reference: update_values = values. Rare
        # (z all-zero); handled on host rather than in the kernel.
        return _np_reference(queries, keys, values, M, z)

    b, s, d = keys.shape
    tot = b * s
    tpc = tot // N_CORES
    nc = _get_kernel(N_CORES, tpc)

    kf = keys.reshape(tot, d)
    vf = values.reshape(tot, d)
    qf = queries.reshape(tot, d)
    z2 = z.reshape(d, 1)

    in_maps = []
    for c in range(N_CORES):
        sl = slice(c * tpc, (c + 1) * tpc)
        in_maps.append(
            {
                "keys": np.ascontiguousarray(kf[sl]),
                "values": np.ascontiguousarray(vf[sl]),
                "queries": np.ascontiguousarray(qf[sl]),
                "m": M,
                "z": z2,
            }
        )

    res = bass_utils.run_bass_kernel_spmd(
        nc, in_maps, core_ids=list(range(N_CORES))
    )
    out = np.concatenate(
        [
            np.asarray(res.results[c]["out"]).astype(np.float32)
            for c in range(N_CORES)
        ],
        axis=0,
    ).reshape(b, s, d)
    if _want_results_obj:
        return out, res
    return out


# revision 22
# speedup vs baseline: 1.0237x; 1.0237x over previous
"""Trainium2 Bass kernel for BaseTensorMemory (delta-rule tensor memory).

Computes, for full inputs queries/keys/values [B,S,D], M [D,D], z [D]:
  sigma_k = elu(keys)+1 ; existing = (sigma_k@M)/(sigma_k@z+eps)
  delta_m = clip(einsum('bsd,bse->de', sigma_k, values-existing)/(B*S), +-1)
  delta_z = sigma_k.sum((0,1))/B
  M' = clip(M+delta_m, +-100); z' = clip(z+delta_z, eps, 1e6)
  out = (sigma_q@M')/(sigma_q@z'+eps)

Strategy: data-parallel over 8 NeuronCores. Flatten B*S tokens, shard
contiguously. Per core: keys pass accumulates [sigma^T@v | sigma^T@1 |
sigma^T@ex] into PSUM, AllReduce the tiny [64,129] partial, build M'/z'
on-chip, then the queries retrieve pass streams the output (bf16).

Engine balance (the point of this version): every [128,1024] bulk op is
placed so ACT / DVE / PE each stay near the ~2.9us/macro-tile DMA floor.
GPSIMD bulk elementwise is ~12x slower than DVE and poisons concurrent
DVE throughput (measured) — it only does tiny memsets, small DMAs, the
casting values->vox DMA (SWDGE descriptor gen) and the collective.
  keys:    ACT exp + 2 sigT copies | DVE min(e,1) (4x) + relu-fold stt +
           ex-mul + recip | PE trans/retr/delta | vox via casting DMA
  queries: ACT exp + relu + 1 sigT copy | DVE min(e,1) + out-mul + recip
           + 1 sigT copy | PE sums min(e,1).T + relu.T via accumulating
           regular matmuls (queries never need token-major sigma)
Other cuts vs the previous version: +eps dropped from the reciprocal
(norm >= ~5, error <= 2e-7), negation folded into the post-allreduce
subtract, output written bf16 (f16 would denorm: out absmax ~3.7e-5).

elu(x)+1 == min(exp(x), relu(x)+1) exactly (e^x >= x+1 everywhere).

Device quirk found empirically: matmuls whose operands alternate base
partition (0 vs 64) inside one PSUM accumulation group hard-crash the
device (NRT_EXEC_UNIT_UNRECOVERABLE). Retrieval is therefore parity-banked:
even token-groups accumulate in bank 0 (operands at base partition 0), odd
groups in bank 1 (base partition 64).
"""

import numpy as np

B, S, D = 16, 16384, 64
N_CORES = 8
EPS = 1e-6
MAX_DELTA = 1.0
MAX_MEMORY = 100.0
MAX_NORM = 1e6

TILE_TOKENS = 2048  # macro-tile: [128, 1024] f32, two 1024-token halves
QPM = TILE_TOKENS // 128  # 16 token-groups per macro-tile
VW = 2 * D + 1  # 129: [v | ones | ex] block per group in VOX


def _build(n_cores, tokens_per_core):
    import concourse.bacc as bacc
    import concourse.mybir as mybir
    import concourse.tile as tile
    from concourse import masks

    dt = mybir.dt
    f32, f16, bf16 = dt.float32, dt.float16, dt.bfloat16
    A = mybir.AluOpType
    F = mybir.ActivationFunctionType

    T = tokens_per_core
    NT = T // TILE_TOKENS
    assert NT * TILE_TOKENS == T
    MFD = QPM * D  # 1024: macro-tile free dim

    nc = bacc.Bacc(
        "TRN2", target_bir_lowering=False, debug=False, num_devices=n_cores
    )
    k_d = nc.dram_tensor("keys", [T, D], f32, kind="ExternalInput").ap()
    v_d = nc.dram_tensor("values", [T, D], f32, kind="ExternalInput").ap()
    q_d = nc.dram_tensor("queries", [T, D], f32, kind="ExternalInput").ap()
    m_d = nc.dram_tensor("m", [D, D], f32, kind="ExternalInput").ap()
    z_d = nc.dram_tensor("z", [D, 1], f32, kind="ExternalInput").ap()
    o_d = nc.dram_tensor("out", [T, D], bf16, kind="ExternalOutput").ap()

    kr = k_d.rearrange("(n p q) d -> n p (q d)", p=128, q=QPM)
    vr = v_d.rearrange("(n p q) d -> n p (q d)", p=128, q=QPM)
    qr = q_d.rearrange("(n p q) d -> n p (q d)", p=128, q=QPM)
    orr = o_d.rearrange("(n p q) d -> n p (q d)", p=128, q=QPM)

    with tile.TileContext(nc) as tc:
        with (
            tc.tile_pool(name="const", bufs=1) as cpool,
            tc.tile_pool(name="io", bufs=4) as io,
            tc.tile_pool(name="qio", bufs=6) as qio,
            tc.tile_pool(name="work", bufs=4) as work,
            tc.tile_pool(name="small", bufs=6) as small,
            tc.tile_pool(name="sigq", bufs=20) as sigq,
            tc.tile_pool(name="psR", bufs=2, space="PSUM") as psRp,
            tc.tile_pool(name="psA", bufs=1, space="PSUM") as psAp,
            tc.tile_pool(name="dram", bufs=1, space="DRAM") as dram,
        ):
            # issue the first macro-tile's loads before anything else so the
            # ACT exp can start as early as possible
            kt0 = io.tile([128, MFD], f32, tag="kt")
            nc.sync.dma_start(kt0[:], kr[0])

            ident = cpool.tile([128, 128], f16)
            masks.make_identity(nc, ident[:])

            # [M|z] in fp16, replicated on both partition halves (parity-
            # banked retrieve uses operands at base partition 0 and 64).
            mz = cpool.tile([128, 65], f32)
            nc.sync.dma_start(mz[0:64, 0:64], m_d[:])
            nc.sync.dma_start(mz[0:64, 64:65], z_d[:])
            nc.sync.dma_start(mz[64:128, 0:64], m_d[:])
            nc.sync.dma_start(mz[64:128, 64:65], z_d[:])
            mz16 = cpool.tile([128, 65], f16)
            nc.scalar.copy(mz16[:], mz[:])

            psA = psAp.tile([64, VW], f32)
            psTh = [None]  # current phase's psT pool

            def elu_k_tile(kt):
                """keys sigma = min(exp(x),1) + relu(x) == elu(x)+1 exactly
                (e^x >= x+1 everywhere). Two DVE ops, no relu pass:
                ep1 = min(e,1) at 4x mode, then one 1x stt folds the relu:
                sig = (x max 0) + ep1. GPSIMD bulk ops are ~12x slower than
                DVE and poison concurrent DVE throughput — measured, avoid."""
                e16 = work.tile([128, MFD], f16, tag="e")
                nc.scalar.activation(e16[:], kt[:], F.Exp)
                ep1 = work.tile([128, MFD], f16, tag="ep1")
                nc.vector.tensor_scalar_min(ep1[:], e16[:], 1.0)
                sig = work.tile([128, MFD], f16, tag="sig")
                nc.vector.scalar_tensor_tensor(
                    sig[:], kt[:], 0.0, ep1[:], op0=A.max, op1=A.add
                )
                return sig

            def elu_q_tile(qt):
                """queries sigma pieces: ep1 = min(exp(x),1) (DVE 4x),
                r16 = relu(x) (ACT). Queries never need token-major sigma,
                only sigma^T — the two pieces are summed for free on the PE
                by accumulating two REGULAR matmuls against identity (the
                is_transpose path can NOT accumulate — it overwrites PSUM
                regardless of start/stop, found empirically — but regular
                matmuls accumulate fine and lhsT.T @ I is a transpose)."""
                e16 = work.tile([128, MFD], f16, tag="e")
                nc.scalar.activation(e16[:], qt[:], F.Exp)
                r16 = work.tile([128, MFD], f16, tag="r")
                nc.scalar.activation(r16[:], qt[:], F.Relu)
                ep1 = work.tile([128, MFD], f16, tag="ep1")
                nc.vector.tensor_scalar_min(ep1[:], e16[:], 1.0)
                return ep1, r16

            def transpose_half(sig, a, copy_eng):
                """4x [128,128] PE transposes of half a -> sigT [128, 512]
                f16 (token-groups parity-stacked on partitions). start=True
                lazily zeroes the whole PSUM bank: first matmul only.
                The psT pool is phase-scoped (keys f16 / queries f32
                pools reuse the same PSUM banks: 3 + 2x2 + 1 = 8)."""
                psT = psTh[0].tile([128, 512], f16, tag="psT")
                for c in range(4):
                    nc.tensor.matmul(
                        psT[:, c * 128 : (c + 1) * 128],
                        sig[:, a * 512 + c * 128 : a * 512 + (c + 1) * 128],
                        ident[:],
                        is_transpose=True,
                        start=(c == 0),
                        stop=(c == 3),
                    )
                sigT = sigq.tile([128, 512], f16, tag="sigT" + copy_eng)
                if copy_eng == "act":
                    nc.scalar.copy(sigT[:], psT[:])
                else:
                    nc.vector.tensor_copy(sigT[:], psT[:])
                return sigT

            def transpose_sum_half(ep1, r16, a, copy_eng):
                """sigT = (ep1 + r16).T via accumulating REGULAR matmuls:
                chunk.T @ I accumulated in PSUM over the two source tiles.
                start=True lazily zeroes the whole PSUM bank: first mm
                only. Regular matmuls must write f32 PSUM."""
                psT = psTh[0].tile([128, 512], f32, tag="psTs")
                for c in range(4):
                    for t_idx, src_t in enumerate((r16, ep1)):
                        nc.tensor.matmul(
                            psT[:, c * 128 : (c + 1) * 128],
                            src_t[:, a * 512 + c * 128 : a * 512 + (c + 1) * 128],
                            ident[:],
                            start=(c == 0 and t_idx == 0),
                            stop=(c == 3 and t_idx == 1),
                        )
                sigT = sigq.tile([128, 512], f16, tag="sigT" + copy_eng)
                if copy_eng == "act":
                    nc.scalar.copy(sigT[:], psT[:])
                else:
                    nc.vector.tensor_copy(sigT[:], psT[:])
                return sigT

            def retrieve_half(sigT, mztile):
                """Parity-banked: even groups -> bank0 (base 0), odd ->
                bank1 (base 64). In-half group (h par, j) = 2j+h at bank h
                col 65j. Returns psum + data/norm views [128, h, j, *]."""
                psR = psRp.tile([128, 1024], f32, tag="psR")
                for g in range(8):
                    par, j = g % 2, g // 2
                    off = par * 512 + 65 * j
                    base = par * 64
                    lhsT = sigT[base : base + 64, j * 128 : (j + 1) * 128]
                    nc.tensor.matmul(
                        psR[:, off : off + 65],
                        lhsT,
                        mztile[base : base + 64, :],
                        start=(j == 0),
                        stop=(j == 3),
                    )
                ret = psR[:].rearrange("p (h x) -> p h x", h=2)[:, :, 0:260]
                ret = ret.rearrange("p h (j c) -> p h j c", j=4)
                return psR, ret[:, :, :, 0:64], ret[:, :, :, 64:65]

            def recip_norm(normv):
                """1/norm broadcast view [128, h, j, 64] f32 (eps dropped:
                norm >= ~5 in practice, error <= 2e-7)."""
                rn = small.tile([128, 8], f32, tag="rn")
                rnv = rn[:].rearrange("p (h j) -> p h j", h=2).unsqueeze(3)
                nc.vector.reciprocal(rnv, normv)
                return rnv.broadcast_to((128, 2, 4, 64))

            # ---------------- keys phase ----------------
            # Stage pipeline over 1024-token halves:
            #   front(h): elu + transposes + sigT copy
            #   mid(h):   retrieve + recip + ex-mul   (1 half behind front)
            #   back(h):  8 delta matmuls             (2 halves behind)
            NH = 2 * NT
            fronts = {}
            first_mm = [True]

            def front_k(h):
                i, a = h // 2, h % 2
                if a == 0:
                    if i == 0:
                        kt = kt0
                    else:
                        # k/v/q arrive f16 via contiguous casting DMAs —
                        # halves SBUF-side DMA traffic; exp(x*(1+2^-11))
                        # keeps sigma within ~0.3% (gate is 2e-2). A strided
                        # casting DMA straight into vox writes 128B chunks —
                        # below the 512B SDMA line-rate threshold — and
                        # saturates the DMA engines (measured); the f16
                        # interleave into vox runs on ACT instead.
                        kt = io.tile([128, MFD], f16, tag="kt")
                        nc.gpsimd.dma_start(kt[:], kr[i])
                    sig = elu_k_tile(kt)
                    vt16 = io.tile([128, MFD], f16, tag="vt16")
                    nc.gpsimd.dma_start(vt16[:], vr[i])
                    vox = work.tile([128, QPM * VW], f16, tag="vox")
                    voxg = vox[:].rearrange("p (g c) -> p g c", g=QPM)
                    nc.scalar.copy(voxg[:, :, 0:64], vt16[:])
                    nc.gpsimd.memset(voxg[:, :, 64:65], 1.0)
                    fronts[i] = (sig, vox)
                sig, vox = fronts[i]
                sigT = transpose_half(sig, a, copy_eng="act")
                return sigT

            def mid_k(h, sigT):
                i, a = h // 2, h % 2
                sig, vox = fronts[i]
                psR, data, normv = retrieve_half(sigT, mz16)
                rn = recip_norm(normv)
                exv = vox[:, a * 8 * VW : (a + 1) * 8 * VW].rearrange(
                    "p (j h c) -> p h j c", j=4, h=2
                )[:, :, :, 65:129]
                nc.vector.tensor_mul(exv, data, rn)

            def back_k(h, last):
                i, a = h // 2, h % 2
                sig, vox = fronts[i]
                for g in range(8):
                    q = a * 8 + g
                    nc.tensor.matmul(
                        psA[:],
                        sig[:, q * 64 : (q + 1) * 64],
                        vox[:, q * VW : (q + 1) * VW],
                        start=first_mm[0],
                        stop=(last and g == 7),
                    )
                    first_mm[0] = False

            # Stage skew of 2 halves between front->mid and mid->back: the
            # cross-engine dependency chain per half (~3-4us) is longer than
            # one half-slot (~1.7us), so a 1-half skew head-of-line-blocks
            # every in-order engine queue (measured: keys phase ran at
            # 5.3us/macro with no engine above ~90%).
            stages = []
            for h in range(NH):
                stages.append(("f", h))
                if h >= 2:
                    stages.append(("m", h - 2))
                if h >= 4:
                    stages.append(("b", h - 4))
            stages += [("m", NH - 2), ("m", NH - 1)]
            stages += [("b", h) for h in range(max(0, NH - 4), NH)]
            sigTs = {}
            with tc.tile_pool(name="psTk", bufs=3, space="PSUM") as psTkp:
                psTh[0] = psTkp
                for kind, h in stages:
                    if kind == "f":
                        sigTs[h] = front_k(h)
                    elif kind == "m":
                        mid_k(h, sigTs[h])
                    else:
                        back_k(h, last=(h == NH - 1))

            # ---------------- allreduce + update ----------------
            accsb = cpool.tile([64, VW], f32)
            nc.vector.tensor_copy(accsb[:], psA[:])
            if n_cores > 1:
                arin = dram.tile([64, VW], f32)
                arout = dram.tile([64, VW], f32)
                nc.gpsimd.dma_start(arin[:], accsb[:])
                nc.gpsimd.collective_compute(
                    "AllReduce",
                    mybir.AluOpType.add,
                    replica_groups=[list(range(n_cores))],
                    ins=[arin.opt()],
                    outs=[arout.opt()],
                )
                arsb = cpool.tile([64, VW], f32)
                nc.gpsimd.dma_start(arsb[:], arout[:])
            else:
                arsb = accsb

            def update_math():
                # 6 fused DVE ops; emitted after SKEW queries fronts so the
                # DVE stream has front work while the AllReduce runs.
                # delta_m = clip((sv-sex)/(B*S), +-1); M' = clip(M+dm, +-100)
                nc.vector.tensor_sub(
                    mzn[:, 0:64], arsb[:, 0:64], arsb[:, 65:129]
                )
                nc.vector.tensor_scalar(
                    mzn[:, 0:64], mzn[:, 0:64], 1.0 / (B * S), MAX_DELTA,
                    op0=A.mult, op1=A.min,
                )
                nc.vector.scalar_tensor_tensor(
                    mzn[:, 0:64], mzn[:, 0:64], -MAX_DELTA, mz[0:64, 0:64],
                    op0=A.max, op1=A.add,
                )
                nc.vector.tensor_scalar(
                    mzn[:, 0:64], mzn[:, 0:64], MAX_MEMORY, -MAX_MEMORY,
                    op0=A.min, op1=A.max,
                )
                # delta_z = acc_z/B; z' = clip(z+dz, eps, 1e6)
                nc.vector.scalar_tensor_tensor(
                    mzn[:, 64:65], arsb[:, 64:65], 1.0 / B, mz[0:64, 64:65],
                    op0=A.mult, op1=A.add,
                )
                nc.vector.tensor_scalar(
                    mzn[:, 64:65], mzn[:, 64:65], EPS, MAX_NORM,
                    op0=A.max, op1=A.min,
                )

            mzn = cpool.tile([64, 65], f32)
            mzn128 = cpool.tile([128, 65], f32)
            mzn16 = cpool.tile([128, 65], f16)

            def update_cast():
                nc.gpsimd.dma_start(mzn128[0:64, :], mzn[:])
                nc.gpsimd.dma_start(mzn128[64:128, :], mzn[:])
                nc.gpsimd.tensor_copy(mzn16[:], mzn128[:])

            # ---------------- queries phase ----------------
            outs = {}

            def front_q(h):
                i, a = h // 2, h % 2
                if a == 0:
                    # f16 casting DMA on the GPSIMD queue: q prefetches are
                    # naturally paced behind the keys-phase loads instead of
                    # stealing keys DMA bandwidth from the sync queue.
                    qt = qio.tile([128, MFD], f16, tag="qt")
                    nc.gpsimd.dma_start(qt[:], qr[i])
                    fronts[i + NT] = elu_q_tile(qt)
                ep1, r16 = fronts[i + NT]
                eng = "act" if a == 0 else "dve"
                return transpose_sum_half(ep1, r16, a, copy_eng=eng)

            def mid_q(h):
                i, a = h // 2, h % 2
                psR, data, normv = retrieve_half(sigTs[h], mzn16)
                rn = recip_norm(normv)
                if a == 0:
                    ot = io.tile([128, MFD], bf16, tag="ot")
                    outs[i] = ot
                ot = outs[i]
                otv = ot[:, a * 512 : (a + 1) * 512].rearrange(
                    "p (j h c) -> p h j c", j=4, h=2
                )
                nc.vector.tensor_mul(otv, data, rn)
                if a == 1:
                    nc.sync.dma_start(orr[i], ot[:])

            SKEW = min(14, NH)
            sigTs = {}
            with tc.tile_pool(name="psTq", bufs=3, space="PSUM") as psTqp:
                psTh[0] = psTqp
                for h in range(min(SKEW, NH)):
                    sigTs[h] = front_q(h)
                update_math()
                update_cast()
                for h in range(SKEW, NH):
                    sigTs[h] = front_q(h)
                    mid_q(h - SKEW)
                for h in range(max(0, NH - SKEW), NH):
                    mid_q(h)

    nc.compile()
    return nc


_CACHE = {}


def _get_kernel(n_cores, tokens_per_core):
    key = (n_cores, tokens_per_core)
    if key not in _CACHE:
        _CACHE[key] = _build(n_cores, tokens_per_core)
    return _CACHE[key]


def _np_reference(queries, keys, values, M, z):
    """Fallback (is_empty edge case) — straight numpy port of the reference."""

    def elu1(x):
        return np.where(x > 0, x + 1.0, np.exp(np.minimum(x, 0.0)))

    def retrieve(sig, M, z):
        return (sig @ M) / ((sig @ z)[..., None] + EPS)

    sk = elu1(keys)
    existing = retrieve(sk, M, z)
    uv = values if z.sum() == 0 else values - existing
    dm = np.clip(
        np.einsum("bsd,bse->de", sk, uv) / (B * S), -MAX_DELTA, MAX_DELTA
    )
    dz = sk.sum(axis=(0, 1)) / B
    Mn = np.clip(M + dm, -MAX_MEMORY, MAX_MEMORY)
    zn = np.clip(z + dz, EPS, MAX_NORM)
    return retrieve(elu1(queries), Mn, zn).astype(np.float32)


def kernel(queries, keys, values, M, z, _want_results_obj=False, **_ignored):
    from concourse import bass_utils

    queries = np.ascontiguousarray(queries, dtype=np.float32)
    keys = np.ascontiguousarray(keys, dtype=np.float32)
    values = np.ascontiguousarray(values, dtype=np.float32)
    M = np.ascontiguousarray(M, dtype=np.float32)
    z = np.ascontiguousarray(z, dtype=np.float32)

    if float(z.sum()) == 0.0:
        # is_empty branch of the reference: update_values = values. Rare
        # (z all-zero); handled on host rather than in the kernel.
        return _np_reference(queries, keys, values, M, z)

    b, s, d = keys.shape
    tot = b * s
    tpc = tot // N_CORES
    nc = _get_kernel(N_CORES, tpc)

    kf = keys.reshape(tot, d)
    vf = values.reshape(tot, d)
    qf = queries.reshape(tot, d)
    z2 = z.reshape(d, 1)

    in_maps = []
    for c in range(N_CORES):
        sl = slice(c * tpc, (c + 1) * tpc)
        in_maps.append(
            {
                "keys": np.ascontiguousarray(kf[sl]),
                "values": np.ascontiguousarray(vf[sl]),
                "queries": np.ascontiguousarray(qf[sl]),
                "m": M,
                "z": z2,
            }
        )

    res = bass_utils.run_bass_kernel_spmd(
        nc, in_maps, core_ids=list(range(N_CORES))
    )
    out = np.concatenate(
        [
            np.asarray(res.results[c]["out"]).astype(np.float32)
            for c in range(N_CORES)
        ],
        axis=0,
    ).reshape(b, s, d)
    if _want_results_obj:
        return out, res
    return out


# revision 23
# speedup vs baseline: 1.2251x; 1.1967x over previous
"""Trainium2 Bass kernel for BaseTensorMemory (delta-rule tensor memory).

Computes, for full inputs queries/keys/values [B,S,D], M [D,D], z [D]:
  sigma_k = elu(keys)+1 ; existing = (sigma_k@M)/(sigma_k@z+eps)
  delta_m = clip(einsum('bsd,bse->de', sigma_k, values-existing)/(B*S), +-1)
  delta_z = sigma_k.sum((0,1))/B
  M' = clip(M+delta_m, +-100); z' = clip(z+delta_z, eps, 1e6)
  out = (sigma_q@M')/(sigma_q@z'+eps)

Strategy: data-parallel over 8 NeuronCores. Flatten B*S tokens, shard
contiguously. Per core: keys pass accumulates [sigma^T@v | sigma^T@1 |
sigma^T@ex] into PSUM, AllReduce the tiny [64,129] partial, build M'/z'
on-chip, then the queries retrieve pass streams the output (bf16).

Engine balance (the point of this version): every [128,1024] bulk op is
placed so ACT / DVE / PE each stay near the ~2.9us/macro-tile DMA floor.
GPSIMD bulk elementwise is ~12x slower than DVE and poisons concurrent
DVE throughput (measured) — it only does tiny memsets, small DMAs, the
casting values->vox DMA (SWDGE descriptor gen) and the collective.
  keys:    ACT exp + 2 sigT copies | DVE min(e,1) (4x) + relu-fold stt +
           ex-mul + recip | PE trans/retr/delta | vox via casting DMA
  queries: ACT exp + relu + 1 sigT copy | DVE min(e,1) + out-mul + recip
           + 1 sigT copy | PE sums min(e,1).T + relu.T via accumulating
           regular matmuls (queries never need token-major sigma)
Other cuts vs the previous version: +eps dropped from the reciprocal
(norm >= ~5, error <= 2e-7), negation folded into the post-allreduce
subtract, output written bf16 (f16 would denorm: out absmax ~3.7e-5).

elu(x)+1 == min(exp(x), relu(x)+1) exactly (e^x >= x+1 everywhere).

Device quirk found empirically: matmuls whose operands alternate base
partition (0 vs 64) inside one PSUM accumulation group hard-crash the
device (NRT_EXEC_UNIT_UNRECOVERABLE). Retrieval is therefore parity-banked:
even token-groups accumulate in bank 0 (operands at base partition 0), odd
groups in bank 1 (base partition 64).
"""

import numpy as np

B, S, D = 16, 16384, 64
N_CORES = 8
EPS = 1e-6
MAX_DELTA = 1.0
MAX_MEMORY = 100.0
MAX_NORM = 1e6

TILE_TOKENS = 2048  # macro-tile: [128, 1024] f32, two 1024-token halves
QPM = TILE_TOKENS // 128  # 16 token-groups per macro-tile
VW = 2 * D + 1  # 129: [v | ones | ex] block per group in VOX


def _build(n_cores, tokens_per_core):
    import concourse.bacc as bacc
    import concourse.mybir as mybir
    import concourse.tile as tile
    from concourse import masks

    dt = mybir.dt
    f32, f16, bf16 = dt.float32, dt.float16, dt.bfloat16
    A = mybir.AluOpType
    F = mybir.ActivationFunctionType

    T = tokens_per_core
    NT = T // TILE_TOKENS
    assert NT * TILE_TOKENS == T
    MFD = QPM * D  # 1024: macro-tile free dim

    nc = bacc.Bacc(
        "TRN2", target_bir_lowering=False, debug=False, num_devices=n_cores
    )
    k_d = nc.dram_tensor("keys", [T, D], f32, kind="ExternalInput").ap()
    v_d = nc.dram_tensor("values", [T, D], f32, kind="ExternalInput").ap()
    q_d = nc.dram_tensor("queries", [T, D], f32, kind="ExternalInput").ap()
    m_d = nc.dram_tensor("m", [D, D], f32, kind="ExternalInput").ap()
    z_d = nc.dram_tensor("z", [D, 1], f32, kind="ExternalInput").ap()
    o_d = nc.dram_tensor("out", [T, D], bf16, kind="ExternalOutput").ap()

    kr = k_d.rearrange("(n p q) d -> n p (q d)", p=128, q=QPM)
    vr = v_d.rearrange("(n p q) d -> n p (q d)", p=128, q=QPM)
    qr = q_d.rearrange("(n p q) d -> n p (q d)", p=128, q=QPM)
    orr = o_d.rearrange("(n p q) d -> n p (q d)", p=128, q=QPM)

    with tile.TileContext(nc) as tc:
        with (
            tc.tile_pool(name="const", bufs=1) as cpool,
            tc.tile_pool(name="io", bufs=4) as io,
            tc.tile_pool(name="qio", bufs=6) as qio,
            tc.tile_pool(name="work", bufs=4) as work,
            tc.tile_pool(name="small", bufs=6) as small,
            tc.tile_pool(name="sigq", bufs=20) as sigq,
            tc.tile_pool(name="psR", bufs=2, space="PSUM") as psRp,
            tc.tile_pool(name="psA", bufs=1, space="PSUM") as psAp,
            tc.tile_pool(name="dram", bufs=1, space="DRAM") as dram,
        ):
            # issue the first macro-tile's loads before anything else so the
            # ACT exp can start as early as possible
            kt0 = io.tile([128, MFD], f32, tag="kt")
            nc.sync.dma_start(kt0[:], kr[0])
            vt0 = io.tile([128, MFD], f32, tag="vt")
            nc.sync.dma_start(vt0[:], vr[0])

            ident = cpool.tile([128, 128], f16)
            masks.make_identity(nc, ident[:])

            # [M|z] in fp16, replicated on both partition halves (parity-
            # banked retrieve uses operands at base partition 0 and 64).
            mz = cpool.tile([128, 65], f32)
            nc.sync.dma_start(mz[0:64, 0:64], m_d[:])
            nc.sync.dma_start(mz[0:64, 64:65], z_d[:])
            nc.sync.dma_start(mz[64:128, 0:64], m_d[:])
            nc.sync.dma_start(mz[64:128, 64:65], z_d[:])
            mz16 = cpool.tile([128, 65], f16)
            nc.scalar.copy(mz16[:], mz[:])

            psA = psAp.tile([64, VW], f32)
            psTh = [None]  # current phase's psT pool

            def elu_k_tile(kt):
                """keys sigma = min(exp(x),1) + relu(x) == elu(x)+1 exactly
                (e^x >= x+1 everywhere). Two DVE ops, no relu pass:
                ep1 = min(e,1) at 4x mode, then one 1x stt folds the relu:
                sig = (x max 0) + ep1. GPSIMD bulk ops are ~12x slower than
                DVE and poison concurrent DVE throughput — measured, avoid."""
                e16 = work.tile([128, MFD], f16, tag="e")
                nc.scalar.activation(e16[:], kt[:], F.Exp)
                ep1 = work.tile([128, MFD], f16, tag="ep1")
                nc.vector.tensor_scalar_min(ep1[:], e16[:], 1.0)
                sig = work.tile([128, MFD], f16, tag="sig")
                nc.vector.scalar_tensor_tensor(
                    sig[:], kt[:], 0.0, ep1[:], op0=A.max, op1=A.add
                )
                return sig

            def elu_q_tile(qt):
                """queries sigma pieces: ep1 = min(exp(x),1) (DVE 4x),
                r16 = relu(x) (ACT). Queries never need token-major sigma,
                only sigma^T — the two pieces are summed for free on the PE
                by accumulating two REGULAR matmuls against identity (the
                is_transpose path can NOT accumulate — it overwrites PSUM
                regardless of start/stop, found empirically — but regular
                matmuls accumulate fine and lhsT.T @ I is a transpose)."""
                e16 = work.tile([128, MFD], f16, tag="e")
                nc.scalar.activation(e16[:], qt[:], F.Exp)
                r16 = work.tile([128, MFD], f16, tag="r")
                nc.scalar.activation(r16[:], qt[:], F.Relu)
                ep1 = work.tile([128, MFD], f16, tag="ep1")
                nc.vector.tensor_scalar_min(ep1[:], e16[:], 1.0)
                return ep1, r16

            def transpose_half(sig, a, copy_eng):
                """4x [128,128] PE transposes of half a -> sigT [128, 512]
                f16 (token-groups parity-stacked on partitions). start=True
                lazily zeroes the whole PSUM bank: first matmul only.
                The psT pool is phase-scoped (keys f16 / queries f32
                pools reuse the same PSUM banks: 3 + 2x2 + 1 = 8)."""
                psT = psTh[0].tile([128, 512], f16, tag="psT")
                for c in range(4):
                    nc.tensor.matmul(
                        psT[:, c * 128 : (c + 1) * 128],
                        sig[:, a * 512 + c * 128 : a * 512 + (c + 1) * 128],
                        ident[:],
                        is_transpose=True,
                        start=(c == 0),
                        stop=(c == 3),
                    )
                sigT = sigq.tile([128, 512], f16, tag="sigT" + copy_eng)
                if copy_eng == "act":
                    nc.scalar.copy(sigT[:], psT[:])
                else:
                    nc.vector.tensor_copy(sigT[:], psT[:])
                return sigT

            def transpose_sum_half(ep1, r16, a, copy_eng):
                """sigT = (ep1 + r16).T via accumulating REGULAR matmuls:
                chunk.T @ I accumulated in PSUM over the two source tiles.
                start=True lazily zeroes the whole PSUM bank: first mm
                only. Regular matmuls must write f32 PSUM."""
                psT = psTh[0].tile([128, 512], f32, tag="psTs")
                for c in range(4):
                    for t_idx, src_t in enumerate((r16, ep1)):
                        nc.tensor.matmul(
                            psT[:, c * 128 : (c + 1) * 128],
                            src_t[:, a * 512 + c * 128 : a * 512 + (c + 1) * 128],
                            ident[:],
                            start=(c == 0 and t_idx == 0),
                            stop=(c == 3 and t_idx == 1),
                        )
                sigT = sigq.tile([128, 512], f16, tag="sigT" + copy_eng)
                if copy_eng == "act":
                    nc.scalar.copy(sigT[:], psT[:])
                else:
                    nc.vector.tensor_copy(sigT[:], psT[:])
                return sigT

            def retrieve_half(sigT, mztile):
                """Parity-banked: even groups -> bank0 (base 0), odd ->
                bank1 (base 64). In-half group (h par, j) = 2j+h at bank h
                col 65j. Returns psum + data/norm views [128, h, j, *]."""
                psR = psRp.tile([128, 1024], f32, tag="psR")
                for g in range(8):
                    par, j = g % 2, g // 2
                    off = par * 512 + 65 * j
                    base = par * 64
                    lhsT = sigT[base : base + 64, j * 128 : (j + 1) * 128]
                    nc.tensor.matmul(
                        psR[:, off : off + 65],
                        lhsT,
                        mztile[base : base + 64, :],
                        start=(j == 0),
                        stop=(j == 3),
                    )
                ret = psR[:].rearrange("p (h x) -> p h x", h=2)[:, :, 0:260]
                ret = ret.rearrange("p h (j c) -> p h j c", j=4)
                return psR, ret[:, :, :, 0:64], ret[:, :, :, 64:65]

            def recip_norm(normv):
                """1/norm broadcast view [128, h, j, 64] f32 (eps dropped:
                norm >= ~5 in practice, error <= 2e-7)."""
                rn = small.tile([128, 8], f32, tag="rn")
                rnv = rn[:].rearrange("p (h j) -> p h j", h=2).unsqueeze(3)
                nc.vector.reciprocal(rnv, normv)
                return rnv.broadcast_to((128, 2, 4, 64))

            # ---------------- keys phase ----------------
            # Stage pipeline over 1024-token halves:
            #   front(h): elu + transposes + sigT copy
            #   mid(h):   retrieve + recip + ex-mul   (1 half behind front)
            #   back(h):  8 delta matmuls             (2 halves behind)
            NH = 2 * NT
            fronts = {}
            first_mm = [True]

            def front_k(h):
                i, a = h // 2, h % 2
                if a == 0:
                    if i == 0:
                        kt = kt0
                    else:
                        # All loads ride the sync/HWDGE path: SWDGE
                        # (gpsimd) casting DMAs added per-device timing
                        # jitter that showed up as 20-35us of inter-device
                        # skew at the AllReduce (measured). A strided
                        # casting DMA straight into vox is also out: 128B
                        # chunks sit below the 512B SDMA line-rate
                        # threshold and saturate the DMA engines.
                        kt = io.tile([128, MFD], f32, tag="kt")
                        nc.sync.dma_start(kt[:], kr[i])
                    sig = elu_k_tile(kt)
                    if i == 0:
                        vt = vt0
                    else:
                        vt = io.tile([128, MFD], f32, tag="vt")
                        nc.sync.dma_start(vt[:], vr[i])
                    vox = work.tile([128, QPM * VW], f16, tag="vox")
                    voxg = vox[:].rearrange("p (g c) -> p g c", g=QPM)
                    nc.scalar.copy(voxg[:, :, 0:64], vt[:])
                    nc.gpsimd.memset(voxg[:, :, 64:65], 1.0)
                    fronts[i] = (sig, vox)
                sig, vox = fronts[i]
                sigT = transpose_half(sig, a, copy_eng="act")
                return sigT

            def mid_k(h, sigT):
                i, a = h // 2, h % 2
                sig, vox = fronts[i]
                psR, data, normv = retrieve_half(sigT, mz16)
                rn = recip_norm(normv)
                exv = vox[:, a * 8 * VW : (a + 1) * 8 * VW].rearrange(
                    "p (j h c) -> p h j c", j=4, h=2
                )[:, :, :, 65:129]
                nc.vector.tensor_mul(exv, data, rn)

            def back_k(h, last):
                i, a = h // 2, h % 2
                sig, vox = fronts[i]
                for g in range(8):
                    q = a * 8 + g
                    nc.tensor.matmul(
                        psA[:],
                        sig[:, q * 64 : (q + 1) * 64],
                        vox[:, q * VW : (q + 1) * VW],
                        start=first_mm[0],
                        stop=(last and g == 7),
                    )
                    first_mm[0] = False

            # Stage skew of 2 halves between front->mid and mid->back: the
            # cross-engine dependency chain per half (~3-4us) is longer than
            # one half-slot (~1.7us), so a 1-half skew head-of-line-blocks
            # every in-order engine queue (measured: keys phase ran at
            # 5.3us/macro with no engine above ~90%).
            stages = []
            for h in range(NH):
                stages.append(("f", h))
                if h >= 2:
                    stages.append(("m", h - 2))
                if h >= 4:
                    stages.append(("b", h - 4))
            stages += [("m", NH - 2), ("m", NH - 1)]
            stages += [("b", h) for h in range(max(0, NH - 4), NH)]
            sigTs = {}
            with tc.tile_pool(name="psTk", bufs=3, space="PSUM") as psTkp:
                psTh[0] = psTkp
                for kind, h in stages:
                    if kind == "f":
                        sigTs[h] = front_k(h)
                    elif kind == "m":
                        mid_k(h, sigTs[h])
                    else:
                        back_k(h, last=(h == NH - 1))

            # ---------------- allreduce + update ----------------
            accsb = cpool.tile([64, VW], f32)
            nc.vector.tensor_copy(accsb[:], psA[:])
            if n_cores > 1:
                arin = dram.tile([64, VW], f32)
                arout = dram.tile([64, VW], f32)
                nc.gpsimd.dma_start(arin[:], accsb[:])
                nc.gpsimd.collective_compute(
                    "AllReduce",
                    mybir.AluOpType.add,
                    replica_groups=[list(range(n_cores))],
                    ins=[arin.opt()],
                    outs=[arout.opt()],
                )
                arsb = cpool.tile([64, VW], f32)
                nc.gpsimd.dma_start(arsb[:], arout[:])
            else:
                arsb = accsb

            def update_math():
                # 6 fused DVE ops; emitted after SKEW queries fronts so the
                # DVE stream has front work while the AllReduce runs.
                # delta_m = clip((sv-sex)/(B*S), +-1); M' = clip(M+dm, +-100)
                nc.vector.tensor_sub(
                    mzn[:, 0:64], arsb[:, 0:64], arsb[:, 65:129]
                )
                nc.vector.tensor_scalar(
                    mzn[:, 0:64], mzn[:, 0:64], 1.0 / (B * S), MAX_DELTA,
                    op0=A.mult, op1=A.min,
                )
                nc.vector.scalar_tensor_tensor(
                    mzn[:, 0:64], mzn[:, 0:64], -MAX_DELTA, mz[0:64, 0:64],
                    op0=A.max, op1=A.add,
                )
                nc.vector.tensor_scalar(
                    mzn[:, 0:64], mzn[:, 0:64], MAX_MEMORY, -MAX_MEMORY,
                    op0=A.min, op1=A.max,
                )
                # delta_z = acc_z/B; z' = clip(z+dz, eps, 1e6)
                nc.vector.scalar_tensor_tensor(
                    mzn[:, 64:65], arsb[:, 64:65], 1.0 / B, mz[0:64, 64:65],
                    op0=A.mult, op1=A.add,
                )
                nc.vector.tensor_scalar(
                    mzn[:, 64:65], mzn[:, 64:65], EPS, MAX_NORM,
                    op0=A.max, op1=A.min,
                )

            mzn = cpool.tile([64, 65], f32)
            mzn128 = cpool.tile([128, 65], f32)
            mzn16 = cpool.tile([128, 65], f16)

            def update_cast():
                nc.gpsimd.dma_start(mzn128[0:64, :], mzn[:])
                nc.gpsimd.dma_start(mzn128[64:128, :], mzn[:])
                nc.gpsimd.tensor_copy(mzn16[:], mzn128[:])

            # ---------------- queries phase ----------------
            outs = {}

            def front_q(h):
                i, a = h // 2, h % 2
                if a == 0:
                    qt = qio.tile([128, MFD], f32, tag="qt")
                    nc.sync.dma_start(qt[:], qr[i])
                    fronts[i + NT] = elu_q_tile(qt)
                ep1, r16 = fronts[i + NT]
                eng = "act" if a == 0 else "dve"
                return transpose_sum_half(ep1, r16, a, copy_eng=eng)

            def mid_q(h):
                i, a = h // 2, h % 2
                psR, data, normv = retrieve_half(sigTs[h], mzn16)
                rn = recip_norm(normv)
                if a == 0:
                    ot = io.tile([128, MFD], bf16, tag="ot")
                    outs[i] = ot
                ot = outs[i]
                otv = ot[:, a * 512 : (a + 1) * 512].rearrange(
                    "p (j h c) -> p h j c", j=4, h=2
                )
                nc.vector.tensor_mul(otv, data, rn)
                if a == 1:
                    nc.sync.dma_start(orr[i], ot[:])

            SKEW = min(14, NH)
            sigTs = {}
            with tc.tile_pool(name="psTq", bufs=3, space="PSUM") as psTqp:
                psTh[0] = psTqp
                for h in range(min(SKEW, NH)):
                    sigTs[h] = front_q(h)
                update_math()
                update_cast()
                for h in range(SKEW, NH):
                    sigTs[h] = front_q(h)
                    mid_q(h - SKEW)
                for h in range(max(0, NH - SKEW), NH):
                    mid_q(h)

    nc.compile()
    return nc


_CACHE = {}


def _get_kernel(n_cores, tokens_per_core):
    key = (n_cores, tokens_per_core)
    if key not in _CACHE:
        _CACHE[key] = _build(n_cores, tokens_per_core)
    return _CACHE[key]


def _np_reference(queries, keys, values, M, z):
    """Fallback (is_empty edge case) — straight numpy port of the reference."""

    def elu1(x):
        return np.where(x > 0, x + 1.0, np.exp(np.minimum(x, 0.0)))

    def retrieve(sig, M, z):
        return (sig @ M) / ((sig @ z)[..., None] + EPS)

    sk = elu1(keys)
    existing = retrieve(sk, M, z)
    uv = values if z.sum() == 0 else values - existing
    dm = np.clip(
        np.einsum("bsd,bse->de", sk, uv) / (B * S), -MAX_DELTA, MAX_DELTA
    )
    dz = sk.sum(axis=(0, 1)) / B
    Mn = np.clip(M + dm, -MAX_MEMORY, MAX_MEMORY)
    zn = np.clip(z + dz, EPS, MAX_NORM)
    return retrieve(elu1(queries), Mn, zn).astype(np.float32)


def kernel(queries, keys, values, M, z, _want_results_obj=False, **_ignored):
    from concourse import bass_utils

    queries = np.ascontiguousarray(queries, dtype=np.float32)
    keys = np.ascontiguousarray(keys, dtype=np.float32)
    values = np.ascontiguousarray(values, dtype=np.float32)
    M = np.ascontiguousarray(M, dtype=np.float32)
    z = np.ascontiguousarray(z, dtype=np.float32)

    if float(z.sum()) == 0.0:
        # is_empty branch of the reference: update_values = values. Rare
        # (z all-zero); handled on host rather than in the kernel.
        return _np_reference(queries, keys, values, M, z)

    b, s, d = keys.shape
    tot = b * s
    tpc = tot // N_CORES
    nc = _get_kernel(N_CORES, tpc)

    kf = keys.reshape(tot, d)
    vf = values.reshape(tot, d)
    qf = queries.reshape(tot, d)
    z2 = z.reshape(d, 1)

    in_maps = []
    for c in range(N_CORES):
        sl = slice(c * tpc, (c + 1) * tpc)
        in_maps.append(
            {
                "keys": np.ascontiguousarray(kf[sl]),
                "values": np.ascontiguousarray(vf[sl]),
                "queries": np.ascontiguousarray(qf[sl]),
                "m": M,
                "z": z2,
            }
        )

    res = bass_utils.run_bass_kernel_spmd(
        nc, in_maps, core_ids=list(range(N_CORES))
    )
    out = np.concatenate(
        [
            np.asarray(res.results[c]["out"]).astype(np.float32)
            for c in range(N_CORES)
        ],
        axis=0,
    ).reshape(b, s, d)
    if _want_results_obj:
        return out, res
    return out
